# revision 65
# baseline (speedup 1.0000x reference)
"""Multi-head causal attention (B=2, S=2048, D=1024, 16 heads x 64) on 8 trn2
NeuronCores.

Sharding: core c = 4*b + g handles batch b and heads [4g, 4g+4) (tensor
parallel over heads, data parallel over batch). Each core:
  - projects q/k/v for its heads from x[b] (wqkv column-sharded by head),
  - applies rotary embeddings,
  - computes causal softmax(q k^T / sqrt(d)) v in a transposed-score layout,
  - multiplies by its shard of wo^T to produce a partial [D, S] output (bf16).
The host sums the 4 head-group partials per batch and transposes.

Device-side layouts (per core):
  xt      [128, 8, 2048]  x[b]^T: partition p + 128*kt = model dim d
  wqkt    [128, 8, 512]   W_{q,k}^T, columns [qE qO kE kO] (evens/odds split
                          per head so RoPE runs as full-width vector ops)
  wvt     [128, 8, 256]   W_v^T, natural head-dim order
  wot     [128, 2, 1024]  wo[:, head cols]^T (matmul stationary)
  cosA/sinA [128, 2048]   rotary tables tiled 4x over the 32 pair dims
  trilm   [128, 128]      upper-triangular 0/1 (valid = key <= query)
  outp    [8, 128, 2048]  partial output, d-major, bf16

Matmul operands are bf16 (half the PE energy of fp32 modes — sustained fp32
work trips the power/activity throttle to half clock — plus fast weight
loads); accumulation is always fp32 in PSUM. RoPE inputs and softmax
normalization stay fp32. Scores/AV matmuls are padded to the full 128x128
array (zero-padded per-head q tiles, over-wide v stationary): half-array
matmuls derate the tensor-engine clock. Softmax skips max-subtraction:
logits are ~N(0,1) for randn-scale inputs, far from fp32 exp overflow.

Engine budget (per core, ~): tensor 145us of matmul streaming, scalar ~84us
of exp — these two are the co-critical engines. Everything else is kept off
them: DMA triggers go to sync (HWDGE) / gpsimd (SWDGE), psum evictions and
memsets to gpsimd, rope + normalization to vector. Input DMAs are issued in
first-needed order at 512-col granularity so the first attention window
starts ~10us in; the output projection runs in two query-halves so its
matmuls and the (bf16) output DMA overlap the attention tail.
"""

import numpy as np
import ml_dtypes

import concourse.bass as bass
import concourse.mybir as mybir
import concourse.tile as tile
from concourse import bacc
from concourse.bass_utils import run_bass_kernel_spmd

N_CORES = 8
B, S, DIM = 2, 2048, 1024
N_HEAD, HD = 16, 64
HPC = N_HEAD // 4  # heads per core = 4
KT = DIM // 128  # 8 contraction tiles over model dim
F32 = mybir.dt.float32
BF16 = mybir.dt.bfloat16
F32R = mybir.dt.float32r
MM_DT = BF16
VROW = HPC * (HD + 1)  # 260: v columns per s-tile (4 heads x [v | ones])
VROWP = VROW + HD - 1  # 323: padded so the 128-wide AV stationary slice
                       # for the last head stays inside its own s-tile row

_programs = {}


def _np_mm_dt(md):
    return ml_dtypes.bfloat16 if md == BF16 else np.float32


def _ranges(start, end, step):
    """[start, end) split at multiples of `step`."""
    out = []
    a = start
    while a < end:
        b = min(end, (a // step + 1) * step)
        out.append((a, b))
        a = b
    return out


def _build_program(causal: bool, taps: bool = False, md=MM_DT):
    nc = bacc.Bacc("TRN2", target_bir_lowering=False, debug=False,
                   num_devices=N_CORES)
    tap_d = {}
    if taps:
        tap_d["yt"] = nc.dram_tensor("yt_dump", [2, 128, S], F32, kind="ExternalOutput")
        tap_d["z"] = nc.dram_tensor("z_dump", [HPC, 2, 1, 1024], F32, kind="ExternalOutput")

    # sc-major x / flat wqk DRAM layouts: input DMAs read 4-8KB contiguous
    # runs per partition instead of 1-2KB strided ones (big throughput win)
    xt_d = nc.dram_tensor("xt", [128, 4, KT, 512], md, kind="ExternalInput")
    wqkt_d = nc.dram_tensor("wqkt", [128, KT * 512], md, kind="ExternalInput")
    wvt_d = nc.dram_tensor("wvt", [128, KT, 256], md, kind="ExternalInput")
    wot_d = nc.dram_tensor("wot", [128, 2, 1024], md, kind="ExternalInput")
    cos_d = nc.dram_tensor("cosA", [128, S], md, kind="ExternalInput")
    sin_d = nc.dram_tensor("sinA", [128, S], md, kind="ExternalInput")
    tril_d = nc.dram_tensor("trilm", [128, 128], md, kind="ExternalInput")
    out_d = nc.dram_tensor("outp", [KT, 128, S], md, kind="ExternalOutput")

    with tile.TileContext(nc) as tc:
      with (
        tc.tile_pool(name="persist", bufs=1) as persist,
        tc.tile_pool(name="pha", bufs=1) as pha,
        tc.tile_pool(name="rope_out", bufs=6) as rope_out,
        tc.tile_pool(name="rope_tmp", bufs=3) as rope_tmp,
        tc.tile_pool(name="attn", bufs=6) as attn_pool,
        tc.tile_pool(name="norm", bufs=3) as norm_pool,
        tc.tile_pool(name="ystage", bufs=2) as ystage,
        tc.tile_pool(name="ostage", bufs=6) as ostage,
      ):
         psS_cm = tc.tile_pool(name="psS", bufs=2, space="PSUM")
         psS = psS_cm.__enter__()
         psY_cm = tc.tile_pool(name="psY", bufs=1, space="PSUM")
         psY = psY_cm.__enter__()
         pp_cm = tc.tile_pool(name="pp", bufs=1, space="PSUM")
         pp = pp_cm.__enter__()

         # qhatA[:, h, :]: head h zero-padded into partitions [(h%2)*64, +64)
         qhatA = persist.tile([128, HPC, S], md, tag="qhatA", name="qhatA")
         khat = [persist.tile([128, S], md, tag=f"khat{t}", name=f"khat{t}") for t in range(2)]
         # v_flat: 16 s-tiles x [4 heads x (v | ones) | zero pad]
         v_sb = persist.tile([128, 16 * VROWP], md, tag="v_sb")
         yt_sb = [persist.tile([128, S], md, tag=f"yt{t}", name=f"yt{t}") for t in range(2)]
         tril_sb = persist.tile([128, 128], md, tag="tril")
         wot = persist.tile([128, 2, 1024], md, tag="wot")
         xt = pha.tile([128, KT, S], md, tag="xt")
         wqk = pha.tile([128, KT, 512], md, tag="wqk")
         wvt = pha.tile([128, KT, 256], md, tag="wvt")
         cosA = pha.tile([128, S], md, tag="cos")
         sinA = pha.tile([128, S], md, tag="sin")
         wz = pha.tile([128, 512], md, tag="wz")
         # warm-up operand with no DMA dependency (vector is idle at start)
         nc.vector.memset(wz[:], 0.0)

         # ---- input DMAs: first-needed first ----------------------------
         # Priority (needed by pair 0 + first attention window, ~10us in):
         # cos/sin + wqk + xt query-half 0, split across the sync and scalar
         # HWDGE queues so both stream concurrently (>=2KB descriptor rows —
         # 1KB rows halve queue throughput). tril/wvt ride gpsimd's SWDGE
         # queue first (land ~9us, needed ~16us), then its memsets, then the
         # bulk second-half loads. Scalar issues nothing after its priority
         # batch: it is the exp engine.
         # Priority inputs (pair 0 + first window) split across FOUR queues —
         # a single queue sustains only ~110GB/s, so the load is parallelized:
         #   sync:   wqk kt0-3 + xt sc0 kt0-3
         #   scalar: sin + wqk kt4-7
         #   vector: cos + xt sc0 kt4-7
         # gpsimd's SWDGE queue takes tril/wvt and the later-needed bulk.
         wqk_src = wqkt_d.ap()[:].rearrange("p (k c) -> p k c", k=KT)
         # no queue carries two chunks gating the same pair-0 matmuls:
         # kt0-3 <- (wqk03 @ sync, xt03 @ scalar); kt4-7 <- both chunks on
         # the faster gpsimd SWDGE queue
         nc.sync.dma_start(out=wqk[:, 0:4, :], in_=wqk_src[:, 0:4])
         nc.scalar.dma_start(out=xt[:, 0:4, 0:512], in_=xt_d.ap()[:, 0, 0:4])
         nc.gpsimd.dma_start(out=xt[:, 4:8, 0:512], in_=xt_d.ap()[:, 0, 4:8])
         nc.gpsimd.dma_start(out=wqk[:, 4:8, :], in_=wqk_src[:, 4:8])
         nc.sync.dma_start(out=sinA[:, 0:1024], in_=sin_d.ap()[:, 0:1024])
         nc.scalar.dma_start(out=cosA[:, 0:1024], in_=cos_d.ap()[:, 0:1024])
         nc.gpsimd.dma_start(out=tril_sb[:], in_=tril_d.ap()[:])
         nc.gpsimd.dma_start(out=wvt[:], in_=wvt_d.ap()[:])
         nc.sync.dma_start(out=xt[:, :, 512:1024], in_=xt_d.ap()[:, 1])

         # gpsimd preamble: zero the q pads (score contraction runs over all
         # 128 partitions; rows outside the head's 64 must stay 0) and the
         # v-tile pads, set the ones column that yields the softmax
         # normalizer through the AV matmul. Slot-0 pad first: the first
         # scores (~18us) contract over it.
         v_rows = v_sb[:, 0:16 * VROWP].rearrange("p (st r) -> p st r", st=16)
         if md == BF16:
             nc.gpsimd.memset(qhatA[64:128, 0:1, :], 0.0)
             nc.gpsimd.memset(qhatA[0:64, 1:2, :], 0.0)
             nc.gpsimd.memset(v_rows[:, :, VROW:VROWP], 0.0)
             nc.gpsimd.memset(v_rows[:, :, HD:VROW:HD + 1], 1.0)
         else:
             raise NotImplementedError("fp32 path removed")
         # bulk second-half loads: land by ~25us, first needed ~35us
         nc.gpsimd.dma_start(out=xt[:, :, 1024:1536], in_=xt_d.ap()[:, 2])
         nc.gpsimd.dma_start(out=xt[:, :, 1536:2048], in_=xt_d.ap()[:, 3])
         if md == BF16:
             nc.gpsimd.memset(qhatA[64:128, 2:3, :], 0.0)
             nc.gpsimd.memset(qhatA[0:64, 3:4, :], 0.0)
         nc.gpsimd.dma_start(out=cosA[:, 1024:2048], in_=cos_d.ap()[:, 1024:2048])
         nc.gpsimd.dma_start(out=sinA[:, 1024:2048], in_=sin_d.ap()[:, 1024:2048])
         nc.gpsimd.dma_start(out=wot[:], in_=wot_d.ap()[:])

         # ---- emission helpers ------------------------------------------
         def emit_warm(n):
             # dummy matmuls with no DMA dependencies: keep the PE busy
             # through input-DMA pacing gaps so the HAM clock gate stays
             # at full speed (idle windows drop the PE to half clock)
             wu = psY.tile([128, 1024], F32, tag="psY", name="wu")
             for i in range(n):
                 nc.tensor.matmul(out=wu[:, 0:512], lhsT=wz[:, 0:128],
                                  rhs=wz[:, 0:512],
                                  start=(i == 0), stop=(i == n - 1))

         def emit_qk_pair(sc, qk, chase=False):
             """Project + rope one (s-chunk, q-or-k) pair of e-tiles."""
             pE = pp.tile([128, 512], F32, tag="ppE", name="ppE")
             pO = pp.tile([128, 512], F32, tag="ppO", name="ppO")
             if chase:
                 # kt-major with dep-free warm matmuls padded in: each input
                 # chunk unlocks both e-tiles' matmuls, and the warms keep
                 # the PE clock ramped through DMA-arrival gaps
                 wu = psY.tile([128, 1024], F32, tag="psY", name="wu")
                 for kt in range(KT):
                     for e, pt in ((2 * qk, pE), (2 * qk + 1, pO)):
                         nc.tensor.matmul(
                             out=pt[:],
                             lhsT=wqk[:, kt, e * 128:(e + 1) * 128],
                             rhs=xt[:, kt, sc * 512:(sc + 1) * 512],
                             start=(kt == 0), stop=(kt == KT - 1),
                             skip_group_check=True,
                         )
                     if kt % 2 == 1 and kt < KT - 1:
                         nc.tensor.matmul(
                             out=wu[:, 0:512], lhsT=wz[:, 0:128],
                             rhs=wz[:, 0:512], start=True, stop=True,
                             skip_group_check=True,
                         )
             else:
                 for e, pt in ((2 * qk, pE), (2 * qk + 1, pO)):
                     for kt in range(KT):
                         nc.tensor.matmul(
                             out=pt[:],
                             lhsT=wqk[:, kt, e * 128:(e + 1) * 128],
                             rhs=xt[:, kt, sc * 512:(sc + 1) * 512],
                             start=(kt == 0), stop=(kt == KT - 1),
                         )
             cs = cosA[:, sc * 512:(sc + 1) * 512]
             sn = sinA[:, sc * 512:(sc + 1) * 512]
             # o2 = [oE | oO]: one tile so the shuffle DMAs read 2KB rows
             o2 = rope_out.tile([128, 1024], md, tag="rope2", name="rope2")
             oE, oO = o2[:, 0:512], o2[:, 512:1024]
             tmp = rope_tmp.tile([128, 512], F32, tag="ropetmp", name="ropetmp")
             # oE = pE*cos - pO*sin ; oO = pO*cos + pE*sin
             nc.vector.tensor_mul(tmp[:], pO[:], sn)
             nc.vector.tensor_mul(oE, pE[:], cs)
             nc.vector.tensor_sub(oE, oE, tmp[:])
             nc.vector.tensor_mul(tmp[:], pE[:], sn)
             nc.vector.tensor_mul(oO, pO[:], cs)
             nc.vector.tensor_add(oO, oO, tmp[:])
             # shuffle rope output into score layout: head h's [E(32)|O(32)]
             # rows go to partitions [(h%2)*64, +64)
             w0, w1 = sc * 512, (sc + 1) * 512
             for h in range(HPC):
                 r0 = (h % 2) * 64
                 if qk == 0:
                     dE = qhatA[r0:r0 + 32, h, w0:w1]
                     dO = qhatA[r0 + 32:r0 + 64, h, w0:w1]
                 else:
                     dE = khat[h // 2][r0:r0 + 32, w0:w1]
                     dO = khat[h // 2][r0 + 32:r0 + 64, w0:w1]
                 nc.sync.dma_start(out=dE, in_=o2[32 * h:32 * h + 32, 0:512])
                 nc.sync.dma_start(out=dO, in_=o2[32 * h:32 * h + 32, 512:1024])

         def emit_v(st):
             pv = pp.tile([128, 256], F32, tag="ppE", name="pv")
             for kt in range(KT):
                 nc.tensor.matmul(
                     out=pv[:],
                     lhsT=xt[:, kt, st * 128:(st + 1) * 128],
                     rhs=wvt[:, kt, :],
                     start=(kt == 0), stop=(kt == KT - 1),
                 )
             vg = v_rows[:, st:st + 1, 0:VROW].rearrange("p st (h d) -> p st h d", h=HPC)
             nc.vector.tensor_copy(vg[:, 0, :, 0:HD],
                                   pv[:].rearrange("p (h d) -> p h d", h=HPC))

         def emit_attn(h, wbase, wlen, pre_k=None, nchunk=1, ytu_eng=None,
                       post_c0=None):
             th, r0 = h // 2, (h % 2) * 64
             py = psY.tile([128, wlen], F32, tag="psY", name="psY")
             kmax = (wbase + wlen) // 128 if causal else 16

             def emit_av(k, at, qs):
                 for (a, b) in _ranges(qs, wlen, 512):
                     c = a // 512
                     if causal:
                         stop_k = min(kmax - 1, (wbase + 512 * (c + 1)) // 128 - 1)
                     else:
                         stop_k = kmax - 1
                     voff = k * VROWP + h * (HD + 1)
                     nc.tensor.matmul(
                         out=py[:, a:b],
                         lhsT=v_sb[:, voff:voff + 128],
                         rhs=at[:, a:b],
                         start=(k == 0), stop=(k == stop_k),
                     )

             for k in range(kmax):
                 qs = max(wbase, 128 * k) - wbase if causal else 0
                 pscore = psS.tile([128, wlen], F32, tag="psS", name="psS")
                 for (a, b) in _ranges(qs, wlen, 512):
                     nc.tensor.matmul(
                         out=pscore[:, a:b],
                         lhsT=khat[th][:, k * 128:(k + 1) * 128],
                         rhs=qhatA[:, h, wbase + a:wbase + b],
                         start=True, stop=True,
                     )
                 at = attn_pool.tile([128, wlen], md, tag="at", name="at")
                 nc.scalar.activation(
                     at[:, qs:wlen], pscore[:, qs:wlen],
                     mybir.ActivationFunctionType.Exp,
                     scale=float(HD) ** -0.5)
                 if causal and 128 * k >= wbase:
                     nc.vector.tensor_mul(
                         at[:, qs:qs + 128], at[:, qs:qs + 128], tril_sb[:])
                 if pre_k is not None:
                     # v for this k emits after the scores: the first score
                     # batch never waits behind v matmuls in tensor order
                     pre_k(k)
                 emit_av(k, at, qs)
             # Evict psum fast, then normalize off the PE critical path.
             # nchunk=2 pipelines the chain for tail windows where its
             # serial latency is exposed.
             cw = wlen // nchunk
             for ci in range(nchunk):
                 co = ci * cw
                 ytu = norm_pool.tile([65, cw], F32, tag="ytu", name="ytu")
                 # the next window's first AV waits on this eviction (psY is
                 # single-buffered); route handoff windows through scalar
                 # when vector is the contended engine
                 if ytu_eng == "scalar":
                     nc.scalar.copy(ytu[:], py[0:65, co:co + cw])
                 else:
                     nc.vector.tensor_copy(ytu[:], py[0:65, co:co + cw])
                 zrow = norm_pool.tile([1, cw], F32, tag="zrow", name="zrow")
                 zb = norm_pool.tile([64, cw], F32, tag="zb", name="zb")
                 zr = norm_pool.tile([64, cw], F32, tag="zr", name="zr")
                 # gpsimd broadcast needs its source at partition 0
                 nc.sync.dma_start(out=zrow[0:1, :], in_=ytu[64:65, :])
                 nc.gpsimd.partition_broadcast(zb[:], zrow[0:1, :])
                 if taps and wlen == 1024 and nchunk == 1:
                     nc.sync.dma_start(out=tap_d["z"].ap()[h, wbase // 1024],
                                       in_=zrow[0:1, :])
                 nc.vector.reciprocal_approx_fast(zr[:], zb[:])
                 wb = wbase + co
                 meng = nc.vector
                 if r0 == 0:
                     meng.tensor_mul(
                         yt_sb[th][0:64, wb:wb + cw], ytu[0:64, :], zr[:])
                 else:
                     yst = ystage.tile([64, cw], md, tag="yst", name="yst")
                     meng.tensor_mul(yst[:], ytu[0:64, :], zr[:])
                     nc.sync.dma_start(
                         out=yt_sb[th][64:128, wb:wb + cw], in_=yst[:])
                 if ci == 0 and post_c0 is not None:
                     post_c0()

         def emit_wo(psO, scs):
             # output projection for query cols [512*scs[0], 512*(scs[-1]+1)):
             # out = wot0^T @ yt0 + wot1^T @ yt1. Only sc==3 runs after the
             # last exp: everything else keeps evictions off scalar (an
             # in-order stall there blocks queued exps).
             for dt in range(KT):
                 ot = ostage.tile([128, 512 * len(scs)], md, tag="ot", name="ot")
                 for i, sc in enumerate(scs):
                     po = psO.tile([128, 512], F32, tag="psO", name="po")
                     for t in range(2):
                         nc.tensor.matmul(
                             out=po[:],
                             lhsT=wot[:, t, dt * 128:(dt + 1) * 128],
                             rhs=yt_sb[t][:, sc * 512:(sc + 1) * 512],
                             start=(t == 0), stop=(t == 1),
                         )
                     a = i * 512
                     if sc == 3 and (dt + sc) % 2 == 1:
                         nc.scalar.copy(ot[:, a:a + 512], po[:])
                     else:
                         nc.vector.tensor_copy(ot[:, a:a + 512], po[:])
                 # spread the staging DMAs over queues: one queue moves
                 # ~110GB/s (~2.3us per 256KB), which otherwise paces ot
                 # recycling
                 if scs[-1] < 2:
                     eng = nc.gpsimd
                 else:
                     eng = (nc.sync, nc.scalar, nc.gpsimd)[dt % 3]
                 eng.dma_start(
                     out=out_d.ap()[dt][:, 512 * scs[0]:512 * (scs[-1] + 1)],
                     in_=ot[:])

         # ---- emission order --------------------------------------------
         if causal:
             # Start attention as early as possible: head 0 runs on 512-wide
             # query windows right after the first two projection pairs;
             # remaining projections interleave under the scalar-engine-bound
             # exp stream. The first output-projection half runs as soon as
             # all heads finish query cols 0:1024, overlapping the attention
             # tail; its output DMA streams out mid-kernel.
             emit_warm(8)
             emit_qk_pair(0, 0, chase=True)
             emit_warm(2)
             emit_qk_pair(0, 1, chase=True)
             emit_attn(0, 0, 512, pre_k=lambda k: emit_v(k) if k < 4 else None)
             emit_qk_pair(1, 0)
             emit_qk_pair(1, 1)
             emit_attn(0, 512, 512,
                       pre_k=lambda k: emit_v(4 + k) if k < 4 else None)
             emit_qk_pair(2, 1)
             emit_attn(1, 0, 1024)
             emit_qk_pair(2, 0)
             emit_attn(2, 0, 1024)
             emit_qk_pair(3, 1)
             emit_qk_pair(3, 0)
             emit_attn(3, 0, 1024)
             emit_attn(1, 1024, 1024,
                       pre_k=lambda k: emit_v(8 + k) if k < 8 else None)
             pp_cm.__exit__(None, None, None)
             psO_cm = tc.tile_pool(name="psO", bufs=2, space="PSUM")
             psO = psO_cm.__enter__()
             emit_attn(2, 1024, 1024)
             emit_wo(psO, (0, 1))
             emit_attn(3, 1024, 1024, nchunk=2)
             emit_attn(0, 1024, 1024, nchunk=2,
                       post_c0=lambda: emit_wo(psO, (2,)))
             psO_cm.__exit__(None, None, None)
             psY_cm.__exit__(None, None, None)
             psS_cm.__exit__(None, None, None)
             with tc.tile_pool(name="psO2", bufs=4, space="PSUM") as psO2:
                 emit_wo(psO2, (3,))
         else:
             for sc in range(4):
                 emit_qk_pair(sc, 0)
                 emit_qk_pair(sc, 1)
             for st in range(16):
                 emit_v(st)
             for h in range(HPC):
                 for j in range(2):
                     emit_attn(h, 1024 * j, 1024)
             pp_cm.__exit__(None, None, None)
             psY_cm.__exit__(None, None, None)
             psS_cm.__exit__(None, None, None)
             with tc.tile_pool(name="psO", bufs=4, space="PSUM") as psO:
                 emit_wo(psO, (0, 1))
                 emit_wo(psO, (2, 3))

         if taps:
             for t in range(2):
                 nc.sync.dma_start(out=tap_d["yt"].ap()[t], in_=yt_sb[t][:])

    nc.compile()
    return nc


def _get_program(causal: bool, md=MM_DT):
    key = (causal, md)
    if key not in _programs:
        _programs[key] = _build_program(causal, md=md)
    return _programs[key]


def _host_prep(x, freqs_cis, wqkv, wo, md=MM_DT):
    """Build per-core device input arrays."""
    nd = _np_mm_dt(md)
    x = np.ascontiguousarray(np.asarray(x, np.float32))
    freqs_cis = np.asarray(freqs_cis, np.float32)
    wqkv = np.asarray(wqkv, np.float32)
    wo = np.asarray(wo, np.float32)

    # x[b]^T in [128, sc, kt, 512] layout (sc-major: contiguous 4KB DMA runs)
    xts = []
    for b in range(B):
        xt = x[b].T  # [DIM, S]
        xt = xt.reshape(KT, 128, S).transpose(1, 0, 2)  # [128, KT, S]
        xt = xt.reshape(128, KT, 4, 512).transpose(0, 2, 1, 3)
        xts.append(np.ascontiguousarray(xt.astype(nd)))

    cosT = np.ascontiguousarray(freqs_cis[:, :, 0].T)  # [32, S]
    sinT = np.ascontiguousarray(freqs_cis[:, :, 1].T)
    cosA = np.ascontiguousarray(np.tile(cosT, (4, 1))).astype(nd)  # [128, S]
    sinA = np.ascontiguousarray(np.tile(sinT, (4, 1))).astype(nd)
    trilm = np.triu(np.ones((128, 128), np.float32)).astype(nd)

    Wq, Wk, Wv = wqkv[0:DIM], wqkv[DIM:2 * DIM], wqkv[2 * DIM:3 * DIM]
    wqk_g, wvt_g, wot_g = [], [], []
    for g in range(4):
        heads = range(4 * g, 4 * g + HPC)
        rows_E = [h * HD + 2 * i for h in heads for i in range(32)]
        rows_O = [h * HD + 2 * i + 1 for h in heads for i in range(32)]
        wqk_shard = np.concatenate(
            [Wq[rows_E], Wq[rows_O], Wk[rows_E], Wk[rows_O]], axis=0)  # [512, DIM]
        wqkt = wqk_shard.T.reshape(KT, 128, 512).transpose(1, 0, 2)
        wqk_g.append(np.ascontiguousarray(
            wqkt.reshape(128, KT * 512).astype(nd)))

        rows_v = [h * HD + d for h in heads for d in range(HD)]
        wvt = Wv[rows_v].T.reshape(KT, 128, 256).transpose(1, 0, 2)
        wvt_g.append(np.ascontiguousarray(wvt.astype(nd)))

        wot = wo[:, rows_v].T.reshape(2, 128, 1024).transpose(1, 0, 2)
        wot_g.append(np.ascontiguousarray(wot.astype(nd)))

    in_maps = []
    for c in range(N_CORES):
        b, g = c // 4, c % 4
        in_maps.append({
            "xt": xts[b], "wqkt": wqk_g[g], "wvt": wvt_g[g], "wot": wot_g[g],
            "cosA": cosA, "sinA": sinA, "trilm": trilm,
        })
    return in_maps


def _host_fallback(x, freqs_cis, mask, wqkv, wo):
    """Generic-mask reference path (numpy, chunked over heads)."""
    x = np.asarray(x, np.float64)
    fc = np.asarray(freqs_cis, np.float64)
    m = np.asarray(mask, bool)[0, 0]
    wqkv64 = np.asarray(wqkv, np.float64)
    wo64 = np.asarray(wo, np.float64)
    qkv = x @ wqkv64.T
    q, k, v = np.split(qkv, 3, axis=-1)
    q = q.reshape(B, S, N_HEAD, HD)
    k = k.reshape(B, S, N_HEAD, HD)
    v = v.reshape(B, S, N_HEAD, HD)

    def rope(t):
        ts = t.reshape(*t.shape[:-1], HD // 2, 2)
        cr = fc[None, :, None, :, 0]
        ci = fc[None, :, None, :, 1]
        xr, xi = ts[..., 0], ts[..., 1]
        return np.stack([xr * cr - xi * ci, xi * cr + xr * ci],
                        axis=-1).reshape(t.shape)

    q, k = rope(q), rope(k)
    out = np.zeros((B, S, DIM), np.float64)
    for h in range(N_HEAD):
        sc = np.einsum("bqd,bkd->bqk", q[:, :, h], k[:, :, h]) * (HD ** -0.5)
        sc = np.where(m[None], sc, -np.inf)
        sc -= sc.max(axis=-1, keepdims=True)
        e = np.exp(sc)
        attn = e / e.sum(axis=-1, keepdims=True)
        y = np.einsum("bqk,bkd->bqd", attn, v[:, :, h])
        out += y @ wo64[:, h * HD:(h + 1) * HD].T
    return out.astype(np.float32)


def kernel(x, freqs_cis, mask, wqkv, wo):
    mask_sq = np.asarray(mask, bool)[0, 0]
    if np.array_equal(mask_sq, np.tril(np.ones((S, S), bool))):
        causal = True
    elif mask_sq.all():
        causal = False
    else:
        return _host_fallback(x, freqs_cis, mask, wqkv, wo)

    # bf16 operands are plenty for genuine rotary tables (cos^2+sin^2=1);
    # free-form freqs widen the logit range beyond bf16 comfort, so take the
    # exact host path for that (not expected in practice).
    fc = np.asarray(freqs_cis, np.float32)
    if not np.allclose(fc[..., 0] ** 2 + fc[..., 1] ** 2, 1.0, atol=0.2):
        return _host_fallback(x, freqs_cis, mask, wqkv, wo)
    md = BF16
    nc = _get_program(causal, md)
    in_maps = _host_prep(x, freqs_cis, wqkv, wo, md)
    res = run_bass_kernel_spmd(nc, in_maps, core_ids=list(range(N_CORES)))

    out = np.zeros((B, S, DIM), np.float32)
    for c in range(N_CORES):
        b = c // 4
        out[b] += res.results[c]["outp"].reshape(DIM, S).T.astype(np.float32)
    return out


# revision 67
# speedup vs baseline: 1.0102x; 1.0102x over previous
"""Multi-head causal attention (B=2, S=2048, D=1024, 16 heads x 64) on 8 trn2
NeuronCores.

Sharding: core c = 4*b + g handles batch b and heads [4g, 4g+4) (tensor
parallel over heads, data parallel over batch). Each core:
  - projects q/k/v for its heads from x[b] (wqkv column-sharded by head),
  - applies rotary embeddings,
  - computes causal softmax(q k^T / sqrt(d)) v in a transposed-score layout,
  - multiplies by its shard of wo^T to produce a partial [D, S] output (bf16).
The host sums the 4 head-group partials per batch and transposes.

Device-side layouts (per core):
  xt      [128, 8, 2048]  x[b]^T: partition p + 128*kt = model dim d
  wqkt    [128, 8, 512]   W_{q,k}^T, columns [qE qO kE kO] (evens/odds split
                          per head so RoPE runs as full-width vector ops)
  wvt     [128, 8, 256]   W_v^T, natural head-dim order
  wot     [128, 2, 1024]  wo[:, head cols]^T (matmul stationary)
  cosA/sinA [128, 2048]   rotary tables tiled 4x over the 32 pair dims
  trilm   [128, 128]      upper-triangular 0/1 (valid = key <= query)
  outp    [8, 128, 2048]  partial output, d-major, bf16

Matmul operands are bf16 (half the PE energy of fp32 modes — sustained fp32
work trips the power/activity throttle to half clock — plus fast weight
loads); accumulation is always fp32 in PSUM. RoPE inputs and softmax
normalization stay fp32. Scores/AV matmuls are padded to the full 128x128
array (zero-padded per-head q tiles, over-wide v stationary): half-array
matmuls derate the tensor-engine clock. Softmax skips max-subtraction:
logits are ~N(0,1) for randn-scale inputs, far from fp32 exp overflow.

Engine budget (per core, ~): tensor 145us of matmul streaming, scalar ~84us
of exp — these two are the co-critical engines. Everything else is kept off
them: DMA triggers go to sync (HWDGE) / gpsimd (SWDGE), psum evictions and
memsets to gpsimd, rope + normalization to vector. Input DMAs are issued in
first-needed order at 512-col granularity so the first attention window
starts ~10us in; the output projection runs in two query-halves so its
matmuls and the (bf16) output DMA overlap the attention tail.
"""

import numpy as np
import ml_dtypes

import concourse.bass as bass
import concourse.mybir as mybir
import concourse.tile as tile
from concourse import bacc
from concourse.bass_utils import run_bass_kernel_spmd

N_CORES = 8
B, S, DIM = 2, 2048, 1024
N_HEAD, HD = 16, 64
HPC = N_HEAD // 4  # heads per core = 4
KT = DIM // 128  # 8 contraction tiles over model dim
F32 = mybir.dt.float32
BF16 = mybir.dt.bfloat16
F32R = mybir.dt.float32r
MM_DT = BF16
VROW = HPC * (HD + 1)  # 260: v columns per s-tile (4 heads x [v | ones])
VROWP = VROW + HD - 1  # 323: padded so the 128-wide AV stationary slice
                       # for the last head stays inside its own s-tile row

_programs = {}


def _np_mm_dt(md):
    return ml_dtypes.bfloat16 if md == BF16 else np.float32


def _ranges(start, end, step):
    """[start, end) split at multiples of `step`."""
    out = []
    a = start
    while a < end:
        b = min(end, (a // step + 1) * step)
        out.append((a, b))
        a = b
    return out


def _build_program(causal: bool, taps: bool = False, md=MM_DT):
    nc = bacc.Bacc("TRN2", target_bir_lowering=False, debug=False,
                   num_devices=N_CORES)
    tap_d = {}
    if taps:
        tap_d["yt"] = nc.dram_tensor("yt_dump", [2, 128, S], F32, kind="ExternalOutput")
        tap_d["z"] = nc.dram_tensor("z_dump", [HPC, 2, 1, 1024], F32, kind="ExternalOutput")

    # sc-major x / flat wqk DRAM layouts: input DMAs read 4-8KB contiguous
    # runs per partition instead of 1-2KB strided ones (big throughput win)
    xt_d = nc.dram_tensor("xt", [128, 4, KT, 512], md, kind="ExternalInput")
    wqkt_d = nc.dram_tensor("wqkt", [128, KT * 512], md, kind="ExternalInput")
    wvt_d = nc.dram_tensor("wvt", [128, KT, 256], md, kind="ExternalInput")
    wot_d = nc.dram_tensor("wot", [128, 2, 1024], md, kind="ExternalInput")
    cos_d = nc.dram_tensor("cosA", [128, S], md, kind="ExternalInput")
    sin_d = nc.dram_tensor("sinA", [128, S], md, kind="ExternalInput")
    tril_d = nc.dram_tensor("trilm", [128, 128], md, kind="ExternalInput")
    out_d = nc.dram_tensor("outp", [KT, 128, S], md, kind="ExternalOutput")

    with tile.TileContext(nc) as tc:
      with (
        tc.tile_pool(name="persist", bufs=1) as persist,
        tc.tile_pool(name="pha", bufs=1) as pha,
        tc.tile_pool(name="rope_out", bufs=6) as rope_out,
        tc.tile_pool(name="rope_tmp", bufs=3) as rope_tmp,
        tc.tile_pool(name="attn", bufs=6) as attn_pool,
        tc.tile_pool(name="norm", bufs=3) as norm_pool,
        tc.tile_pool(name="ystage", bufs=2) as ystage,
        tc.tile_pool(name="ostage", bufs=6) as ostage,
      ):
         psS_cm = tc.tile_pool(name="psS", bufs=2, space="PSUM")
         psS = psS_cm.__enter__()
         psY_cm = tc.tile_pool(name="psY", bufs=1, space="PSUM")
         psY = psY_cm.__enter__()
         pp_cm = tc.tile_pool(name="pp", bufs=1, space="PSUM")
         pp = pp_cm.__enter__()

         # qhatA[:, h, :]: head h zero-padded into partitions [(h%2)*64, +64)
         qhatA = persist.tile([128, HPC, S], md, tag="qhatA", name="qhatA")
         khat = [persist.tile([128, S], md, tag=f"khat{t}", name=f"khat{t}") for t in range(2)]
         # v_flat: 16 s-tiles x [4 heads x (v | ones) | zero pad]
         v_sb = persist.tile([128, 16 * VROWP], md, tag="v_sb")
         yt_sb = [persist.tile([128, S], md, tag=f"yt{t}", name=f"yt{t}") for t in range(2)]
         tril_sb = persist.tile([128, 128], md, tag="tril")
         wot = persist.tile([128, 2, 1024], md, tag="wot")
         xt = pha.tile([128, KT, S], md, tag="xt")
         wqk = pha.tile([128, KT, 512], md, tag="wqk")
         wvt = pha.tile([128, KT, 256], md, tag="wvt")
         cosA = pha.tile([128, S], md, tag="cos")
         sinA = pha.tile([128, S], md, tag="sin")
         wz = pha.tile([128, 512], md, tag="wz")
         # warm-up operand with no DMA dependency (vector is idle at start)
         nc.vector.memset(wz[:], 0.0)

         # ---- input DMAs: first-needed first ----------------------------
         # Priority (needed by pair 0 + first attention window, ~10us in):
         # cos/sin + wqk + xt query-half 0, split across the sync and scalar
         # HWDGE queues so both stream concurrently (>=2KB descriptor rows —
         # 1KB rows halve queue throughput). tril/wvt ride gpsimd's SWDGE
         # queue first (land ~9us, needed ~16us), then its memsets, then the
         # bulk second-half loads. Scalar issues nothing after its priority
         # batch: it is the exp engine.
         # Priority inputs (pair 0 + first window) split across FOUR queues —
         # a single queue sustains only ~110GB/s, so the load is parallelized:
         #   sync:   wqk kt0-3 + xt sc0 kt0-3
         #   scalar: sin + wqk kt4-7
         #   vector: cos + xt sc0 kt4-7
         # gpsimd's SWDGE queue takes tril/wvt and the later-needed bulk.
         wqk_src = wqkt_d.ap()[:].rearrange("p (k c) -> p k c", k=KT)
         # no queue carries two chunks gating the same pair-0 matmuls:
         # kt0-3 <- (wqk03 @ sync, xt03 @ scalar); kt4-7 <- both chunks on
         # the faster gpsimd SWDGE queue
         nc.sync.dma_start(out=wqk[:, 0:4, :], in_=wqk_src[:, 0:4])
         nc.scalar.dma_start(out=xt[:, 0:4, 0:512], in_=xt_d.ap()[:, 0, 0:4])
         nc.gpsimd.dma_start(out=xt[:, 4:8, 0:512], in_=xt_d.ap()[:, 0, 4:8])
         nc.gpsimd.dma_start(out=wqk[:, 4:8, :], in_=wqk_src[:, 4:8])
         nc.sync.dma_start(out=sinA[:, 0:1024], in_=sin_d.ap()[:, 0:1024])
         nc.scalar.dma_start(out=cosA[:, 0:1024], in_=cos_d.ap()[:, 0:1024])
         nc.gpsimd.dma_start(out=tril_sb[:], in_=tril_d.ap()[:])
         nc.gpsimd.dma_start(out=wvt[:], in_=wvt_d.ap()[:])
         nc.sync.dma_start(out=xt[:, :, 512:1024], in_=xt_d.ap()[:, 1])

         # gpsimd preamble: zero the q pads (score contraction runs over all
         # 128 partitions; rows outside the head's 64 must stay 0) and the
         # v-tile pads, set the ones column that yields the softmax
         # normalizer through the AV matmul. Slot-0 pad first: the first
         # scores (~18us) contract over it.
         v_rows = v_sb[:, 0:16 * VROWP].rearrange("p (st r) -> p st r", st=16)
         if md == BF16:
             nc.gpsimd.memset(qhatA[64:128, 0:1, :], 0.0)
             nc.gpsimd.memset(qhatA[0:64, 1:2, :], 0.0)
             nc.gpsimd.memset(v_rows[:, :, VROW:VROWP], 0.0)
             nc.gpsimd.memset(v_rows[:, :, HD:VROW:HD + 1], 1.0)
         else:
             raise NotImplementedError("fp32 path removed")
         # bulk second-half loads: land by ~25us, first needed ~35us
         nc.gpsimd.dma_start(out=xt[:, :, 1024:1536], in_=xt_d.ap()[:, 2])
         nc.gpsimd.dma_start(out=xt[:, :, 1536:2048], in_=xt_d.ap()[:, 3])
         if md == BF16:
             nc.gpsimd.memset(qhatA[64:128, 2:3, :], 0.0)
             nc.gpsimd.memset(qhatA[0:64, 3:4, :], 0.0)
         nc.gpsimd.dma_start(out=cosA[:, 1024:2048], in_=cos_d.ap()[:, 1024:2048])
         nc.gpsimd.dma_start(out=sinA[:, 1024:2048], in_=sin_d.ap()[:, 1024:2048])
         nc.gpsimd.dma_start(out=wot[:], in_=wot_d.ap()[:])

         # ---- emission helpers ------------------------------------------
         def emit_warm(n):
             # dummy matmuls with no DMA dependencies: keep the PE busy
             # through input-DMA pacing gaps so the HAM clock gate stays
             # at full speed (idle windows drop the PE to half clock)
             wu = psY.tile([128, 1024], F32, tag="psY", name="wu")
             for i in range(n):
                 nc.tensor.matmul(out=wu[:, 0:512], lhsT=wz[:, 0:128],
                                  rhs=wz[:, 0:512],
                                  start=(i == 0), stop=(i == n - 1))

         def emit_qk_pair(sc, qk, chase=False):
             """Project + rope one (s-chunk, q-or-k) pair of e-tiles."""
             pE = pp.tile([128, 512], F32, tag="ppE", name="ppE")
             pO = pp.tile([128, 512], F32, tag="ppO", name="ppO")
             if chase:
                 # kt-major with dep-free warm matmuls padded in: each input
                 # chunk unlocks both e-tiles' matmuls, and the warms keep
                 # the PE clock ramped through DMA-arrival gaps
                 wu = psY.tile([128, 1024], F32, tag="psY", name="wu")
                 for kt in range(KT):
                     for e, pt in ((2 * qk, pE), (2 * qk + 1, pO)):
                         nc.tensor.matmul(
                             out=pt[:],
                             lhsT=wqk[:, kt, e * 128:(e + 1) * 128],
                             rhs=xt[:, kt, sc * 512:(sc + 1) * 512],
                             start=(kt == 0), stop=(kt == KT - 1),
                             skip_group_check=True,
                         )
                     if kt % 2 == 1 and kt < KT - 1:
                         nc.tensor.matmul(
                             out=wu[:, 0:512], lhsT=wz[:, 0:128],
                             rhs=wz[:, 0:512], start=True, stop=True,
                             skip_group_check=True,
                         )
             else:
                 for e, pt in ((2 * qk, pE), (2 * qk + 1, pO)):
                     for kt in range(KT):
                         nc.tensor.matmul(
                             out=pt[:],
                             lhsT=wqk[:, kt, e * 128:(e + 1) * 128],
                             rhs=xt[:, kt, sc * 512:(sc + 1) * 512],
                             start=(kt == 0), stop=(kt == KT - 1),
                         )
             cs = cosA[:, sc * 512:(sc + 1) * 512]
             sn = sinA[:, sc * 512:(sc + 1) * 512]
             # o2 = [oE | oO]: one tile so the shuffle DMAs read 2KB rows
             o2 = rope_out.tile([128, 1024], md, tag="rope2", name="rope2")
             oE, oO = o2[:, 0:512], o2[:, 512:1024]
             tmp = rope_tmp.tile([128, 512], F32, tag="ropetmp", name="ropetmp")
             # oE = pE*cos - pO*sin ; oO = pO*cos + pE*sin
             nc.vector.tensor_mul(tmp[:], pO[:], sn)
             nc.vector.tensor_mul(oE, pE[:], cs)
             nc.vector.tensor_sub(oE, oE, tmp[:])
             nc.vector.tensor_mul(tmp[:], pE[:], sn)
             nc.vector.tensor_mul(oO, pO[:], cs)
             nc.vector.tensor_add(oO, oO, tmp[:])
             # shuffle rope output into score layout: head h's [E(32)|O(32)]
             # rows go to partitions [(h%2)*64, +64)
             w0, w1 = sc * 512, (sc + 1) * 512
             for h in range(HPC):
                 r0 = (h % 2) * 64
                 if qk == 0:
                     dE = qhatA[r0:r0 + 32, h, w0:w1]
                     dO = qhatA[r0 + 32:r0 + 64, h, w0:w1]
                 else:
                     dE = khat[h // 2][r0:r0 + 32, w0:w1]
                     dO = khat[h // 2][r0 + 32:r0 + 64, w0:w1]
                 nc.sync.dma_start(out=dE, in_=o2[32 * h:32 * h + 32, 0:512])
                 nc.sync.dma_start(out=dO, in_=o2[32 * h:32 * h + 32, 512:1024])

         def emit_v(st):
             pv = pp.tile([128, 256], F32, tag="ppE", name="pv")
             for kt in range(KT):
                 nc.tensor.matmul(
                     out=pv[:],
                     lhsT=xt[:, kt, st * 128:(st + 1) * 128],
                     rhs=wvt[:, kt, :],
                     start=(kt == 0), stop=(kt == KT - 1),
                 )
             vg = v_rows[:, st:st + 1, 0:VROW].rearrange("p st (h d) -> p st h d", h=HPC)
             nc.vector.tensor_copy(vg[:, 0, :, 0:HD],
                                   pv[:].rearrange("p (h d) -> p h d", h=HPC))

         def emit_attn(h, wbase, wlen, pre_k=None, nchunk=1, ytu_eng=None,
                       post_c0=None):
             th, r0 = h // 2, (h % 2) * 64
             py = psY.tile([128, wlen], F32, tag="psY", name="psY")
             kmax = (wbase + wlen) // 128 if causal else 16

             def emit_av(k, at, qs):
                 for (a, b) in _ranges(qs, wlen, 512):
                     c = a // 512
                     if causal:
                         stop_k = min(kmax - 1, (wbase + 512 * (c + 1)) // 128 - 1)
                     else:
                         stop_k = kmax - 1
                     voff = k * VROWP + h * (HD + 1)
                     nc.tensor.matmul(
                         out=py[:, a:b],
                         lhsT=v_sb[:, voff:voff + 128],
                         rhs=at[:, a:b],
                         start=(k == 0), stop=(k == stop_k),
                     )

             for k in range(kmax):
                 qs = max(wbase, 128 * k) - wbase if causal else 0
                 pscore = psS.tile([128, wlen], F32, tag="psS", name="psS")
                 for (a, b) in _ranges(qs, wlen, 512):
                     nc.tensor.matmul(
                         out=pscore[:, a:b],
                         lhsT=khat[th][:, k * 128:(k + 1) * 128],
                         rhs=qhatA[:, h, wbase + a:wbase + b],
                         start=True, stop=True,
                     )
                 at = attn_pool.tile([128, wlen], md, tag="at", name="at")
                 nc.scalar.activation(
                     at[:, qs:wlen], pscore[:, qs:wlen],
                     mybir.ActivationFunctionType.Exp,
                     scale=float(HD) ** -0.5)
                 if causal and 128 * k >= wbase:
                     nc.vector.tensor_mul(
                         at[:, qs:qs + 128], at[:, qs:qs + 128], tril_sb[:])
                 if pre_k is not None:
                     # v for this k emits after the scores: the first score
                     # batch never waits behind v matmuls in tensor order
                     pre_k(k)
                 emit_av(k, at, qs)
             # Evict psum fast, then normalize off the PE critical path.
             # nchunk=2 pipelines the chain for tail windows where its
             # serial latency is exposed.
             cw = wlen // nchunk
             for ci in range(nchunk):
                 co = ci * cw
                 ytu = norm_pool.tile([65, cw], F32, tag="ytu", name="ytu")
                 # the next window's first AV waits on this eviction (psY is
                 # single-buffered); route handoff windows through scalar
                 # when vector is the contended engine
                 if ytu_eng == "scalar":
                     nc.scalar.copy(ytu[:], py[0:65, co:co + cw])
                 else:
                     nc.vector.tensor_copy(ytu[:], py[0:65, co:co + cw])
                 zrow = norm_pool.tile([1, cw], F32, tag="zrow", name="zrow")
                 zb = norm_pool.tile([64, cw], F32, tag="zb", name="zb")
                 zr = norm_pool.tile([64, cw], F32, tag="zr", name="zr")
                 # gpsimd broadcast needs its source at partition 0
                 nc.sync.dma_start(out=zrow[0:1, :], in_=ytu[64:65, :])
                 nc.gpsimd.partition_broadcast(zb[:], zrow[0:1, :])
                 if taps and wlen == 1024 and nchunk == 1:
                     nc.sync.dma_start(out=tap_d["z"].ap()[h, wbase // 1024],
                                       in_=zrow[0:1, :])
                 nc.vector.reciprocal_approx_fast(zr[:], zb[:])
                 wb = wbase + co
                 meng = nc.vector
                 if r0 == 0:
                     meng.tensor_mul(
                         yt_sb[th][0:64, wb:wb + cw], ytu[0:64, :], zr[:])
                 else:
                     yst = ystage.tile([64, cw], md, tag="yst", name="yst")
                     meng.tensor_mul(yst[:], ytu[0:64, :], zr[:])
                     nc.sync.dma_start(
                         out=yt_sb[th][64:128, wb:wb + cw], in_=yst[:])
                 if ci == 0 and post_c0 is not None:
                     post_c0()

         def emit_wo(psO, scs):
             # output projection for query cols [512*scs[0], 512*(scs[-1]+1)):
             # out = wot0^T @ yt0 + wot1^T @ yt1. Only sc==3 runs after the
             # last exp: everything else keeps evictions off scalar (an
             # in-order stall there blocks queued exps).
             for dt in range(KT):
                 ot = ostage.tile([128, 512 * len(scs)], md, tag="ot", name="ot")
                 for i, sc in enumerate(scs):
                     po = psO.tile([128, 512], F32, tag="psO", name="po")
                     for t in range(2):
                         nc.tensor.matmul(
                             out=po[:],
                             lhsT=wot[:, t, dt * 128:(dt + 1) * 128],
                             rhs=yt_sb[t][:, sc * 512:(sc + 1) * 512],
                             start=(t == 0), stop=(t == 1),
                         )
                     a = i * 512
                     if sc >= 2 and (dt + sc) % 2 == 1:
                         nc.scalar.copy(ot[:, a:a + 512], po[:])
                     else:
                         nc.vector.tensor_copy(ot[:, a:a + 512], po[:])
                 # spread the staging DMAs over queues: one queue moves
                 # ~110GB/s (~2.3us per 256KB), which otherwise paces ot
                 # recycling
                 if scs[-1] < 2:
                     eng = nc.gpsimd
                 else:
                     eng = (nc.sync, nc.scalar, nc.gpsimd)[dt % 3]
                 eng.dma_start(
                     out=out_d.ap()[dt][:, 512 * scs[0]:512 * (scs[-1] + 1)],
                     in_=ot[:])

         # ---- emission order --------------------------------------------
         if causal:
             # Start attention as early as possible: head 0 runs on 512-wide
             # query windows right after the first two projection pairs;
             # remaining projections interleave under the scalar-engine-bound
             # exp stream. The first output-projection half runs as soon as
             # all heads finish query cols 0:1024, overlapping the attention
             # tail; its output DMA streams out mid-kernel.
             emit_warm(8)
             emit_qk_pair(0, 0, chase=True)
             emit_warm(2)
             emit_qk_pair(0, 1, chase=True)
             emit_attn(0, 0, 512, pre_k=lambda k: emit_v(k) if k < 4 else None)
             emit_qk_pair(1, 0)
             emit_qk_pair(1, 1)
             emit_attn(0, 512, 512,
                       pre_k=lambda k: emit_v(4 + k) if k < 4 else None)
             emit_qk_pair(2, 1)
             emit_attn(1, 0, 1024)
             emit_qk_pair(2, 0)
             emit_attn(2, 0, 1024)
             emit_qk_pair(3, 1)
             emit_qk_pair(3, 0)
             emit_attn(3, 0, 1024)
             emit_attn(1, 1024, 1024,
                       pre_k=lambda k: emit_v(8 + k) if k < 8 else None)
             pp_cm.__exit__(None, None, None)
             psO_cm = tc.tile_pool(name="psO", bufs=2, space="PSUM")
             psO = psO_cm.__enter__()
             emit_attn(2, 1024, 1024)
             emit_wo(psO, (0, 1))
             emit_attn(3, 1024, 1024, nchunk=2)
             emit_attn(0, 1024, 1024, nchunk=2)
             psO_cm.__exit__(None, None, None)
             psY_cm.__exit__(None, None, None)
             psS_cm.__exit__(None, None, None)
             with tc.tile_pool(name="psO2", bufs=4, space="PSUM") as psO2:
                 emit_wo(psO2, (2, 3))
         else:
             for sc in range(4):
                 emit_qk_pair(sc, 0)
                 emit_qk_pair(sc, 1)
             for st in range(16):
                 emit_v(st)
             for h in range(HPC):
                 for j in range(2):
                     emit_attn(h, 1024 * j, 1024)
             pp_cm.__exit__(None, None, None)
             psY_cm.__exit__(None, None, None)
             psS_cm.__exit__(None, None, None)
             with tc.tile_pool(name="psO", bufs=4, space="PSUM") as psO:
                 emit_wo(psO, (0, 1))
                 emit_wo(psO, (2, 3))

         if taps:
             for t in range(2):
                 nc.sync.dma_start(out=tap_d["yt"].ap()[t], in_=yt_sb[t][:])

    nc.compile()
    return nc


def _get_program(causal: bool, md=MM_DT):
    key = (causal, md)
    if key not in _programs:
        _programs[key] = _build_program(causal, md=md)
    return _programs[key]


def _host_prep(x, freqs_cis, wqkv, wo, md=MM_DT):
    """Build per-core device input arrays."""
    nd = _np_mm_dt(md)
    x = np.ascontiguousarray(np.asarray(x, np.float32))
    freqs_cis = np.asarray(freqs_cis, np.float32)
    wqkv = np.asarray(wqkv, np.float32)
    wo = np.asarray(wo, np.float32)

    # x[b]^T in [128, sc, kt, 512] layout (sc-major: contiguous 4KB DMA runs)
    xts = []
    for b in range(B):
        xt = x[b].T  # [DIM, S]
        xt = xt.reshape(KT, 128, S).transpose(1, 0, 2)  # [128, KT, S]
        xt = xt.reshape(128, KT, 4, 512).transpose(0, 2, 1, 3)
        xts.append(np.ascontiguousarray(xt.astype(nd)))

    cosT = np.ascontiguousarray(freqs_cis[:, :, 0].T)  # [32, S]
    sinT = np.ascontiguousarray(freqs_cis[:, :, 1].T)
    cosA = np.ascontiguousarray(np.tile(cosT, (4, 1))).astype(nd)  # [128, S]
    sinA = np.ascontiguousarray(np.tile(sinT, (4, 1))).astype(nd)
    trilm = np.triu(np.ones((128, 128), np.float32)).astype(nd)

    Wq, Wk, Wv = wqkv[0:DIM], wqkv[DIM:2 * DIM], wqkv[2 * DIM:3 * DIM]
    wqk_g, wvt_g, wot_g = [], [], []
    for g in range(4):
        heads = range(4 * g, 4 * g + HPC)
        rows_E = [h * HD + 2 * i for h in heads for i in range(32)]
        rows_O = [h * HD + 2 * i + 1 for h in heads for i in range(32)]
        wqk_shard = np.concatenate(
            [Wq[rows_E], Wq[rows_O], Wk[rows_E], Wk[rows_O]], axis=0)  # [512, DIM]
        wqkt = wqk_shard.T.reshape(KT, 128, 512).transpose(1, 0, 2)
        wqk_g.append(np.ascontiguousarray(
            wqkt.reshape(128, KT * 512).astype(nd)))

        rows_v = [h * HD + d for h in heads for d in range(HD)]
        wvt = Wv[rows_v].T.reshape(KT, 128, 256).transpose(1, 0, 2)
        wvt_g.append(np.ascontiguousarray(wvt.astype(nd)))

        wot = wo[:, rows_v].T.reshape(2, 128, 1024).transpose(1, 0, 2)
        wot_g.append(np.ascontiguousarray(wot.astype(nd)))

    in_maps = []
    for c in range(N_CORES):
        b, g = c // 4, c % 4
        in_maps.append({
            "xt": xts[b], "wqkt": wqk_g[g], "wvt": wvt_g[g], "wot": wot_g[g],
            "cosA": cosA, "sinA": sinA, "trilm": trilm,
        })
    return in_maps


def _host_fallback(x, freqs_cis, mask, wqkv, wo):
    """Generic-mask reference path (numpy, chunked over heads)."""
    x = np.asarray(x, np.float64)
    fc = np.asarray(freqs_cis, np.float64)
    m = np.asarray(mask, bool)[0, 0]
    wqkv64 = np.asarray(wqkv, np.float64)
    wo64 = np.asarray(wo, np.float64)
    qkv = x @ wqkv64.T
    q, k, v = np.split(qkv, 3, axis=-1)
    q = q.reshape(B, S, N_HEAD, HD)
    k = k.reshape(B, S, N_HEAD, HD)
    v = v.reshape(B, S, N_HEAD, HD)

    def rope(t):
        ts = t.reshape(*t.shape[:-1], HD // 2, 2)
        cr = fc[None, :, None, :, 0]
        ci = fc[None, :, None, :, 1]
        xr, xi = ts[..., 0], ts[..., 1]
        return np.stack([xr * cr - xi * ci, xi * cr + xr * ci],
                        axis=-1).reshape(t.shape)

    q, k = rope(q), rope(k)
    out = np.zeros((B, S, DIM), np.float64)
    for h in range(N_HEAD):
        sc = np.einsum("bqd,bkd->bqk", q[:, :, h], k[:, :, h]) * (HD ** -0.5)
        sc = np.where(m[None], sc, -np.inf)
        sc -= sc.max(axis=-1, keepdims=True)
        e = np.exp(sc)
        attn = e / e.sum(axis=-1, keepdims=True)
        y = np.einsum("bqk,bkd->bqd", attn, v[:, :, h])
        out += y @ wo64[:, h * HD:(h + 1) * HD].T
    return out.astype(np.float32)


def kernel(x, freqs_cis, mask, wqkv, wo):
    mask_sq = np.asarray(mask, bool)[0, 0]
    if np.array_equal(mask_sq, np.tril(np.ones((S, S), bool))):
        causal = True
    elif mask_sq.all():
        causal = False
    else:
        return _host_fallback(x, freqs_cis, mask, wqkv, wo)

    # bf16 operands are plenty for genuine rotary tables (cos^2+sin^2=1);
    # free-form freqs widen the logit range beyond bf16 comfort, so take the
    # exact host path for that (not expected in practice).
    fc = np.asarray(freqs_cis, np.float32)
    if not np.allclose(fc[..., 0] ** 2 + fc[..., 1] ** 2, 1.0, atol=0.2):
        return _host_fallback(x, freqs_cis, mask, wqkv, wo)
    md = BF16
    nc = _get_program(causal, md)
    in_maps = _host_prep(x, freqs_cis, wqkv, wo, md)
    res = run_bass_kernel_spmd(nc, in_maps, core_ids=list(range(N_CORES)))

    out = np.zeros((B, S, DIM), np.float32)
    for c in range(N_CORES):
        b = c // 4
        out[b] += res.results[c]["outp"].reshape(DIM, S).T.astype(np.float32)
    return out
